# revision 1
# baseline (speedup 1.0000x reference)
"""Trainium2 Bass kernel for ConditionalNeuralNetwork (MoE-style routed MLP).

Strategy (expert-parallel over combos, data-parallel within a combo):
  - Host computes combo idx = 2*flags[:,0] + flags[:,1] per row, groups rows
    by combo, and splits each combo's rows across 2 of the 8 cores.
  - Each core receives only ITS head's weights, so the device kernel is a
    plain dense MLP 256 -> 1024 -> 1024 -> 512 -> 256 -> 1 with relu/sigmoid.
    This halves head FLOPs vs computing all 4 heads densely.
  - Matmul inputs are bf16 (full PE rate, FWL weight loads); accumulation
    and bias+activation epilogues are fp32 in PSUM/ACT.
  - Host scatters per-core outputs back to original row order.
"""

import os
import sys

import ml_dtypes
import numpy as np

for _p in ("/opt/trn_rl_repo", "/root/.axon_site/_ro/trn_rl_repo"):
    if os.path.isdir(_p) and _p not in sys.path:
        sys.path.append(_p)

import concourse.bacc as bacc
import concourse.bass as bass
import concourse.tile as tile
from concourse import mybir
from concourse.bass import MemorySpace
from concourse.bass_utils import run_bass_kernel_spmd

F32 = mybir.dt.float32
BF16 = mybir.dt.bfloat16
AF = mybir.ActivationFunctionType
NPBF16 = ml_dtypes.bfloat16

B, D_IN = 16384, 256
S1, S2 = 1024, 1024
H1, H2 = 512, 256
C = 4
NCORES = 8
N_CHUNKS = 5
CAP = 2080  # rows per core (max needed with seed-0 counts: 2080)
CHUNK = CAP // N_CHUNKS  # 416: keeps every matmul MM-bound (not LDW-bound)
WARMUP_MMS = 5  # dependency-free PE warm-up matmuls at kernel start

_nc_cache = {}
_last_results = None


def _build(cap=CAP):
    """Build the single-core MLP program (SPMD across 8 cores)."""
    nc = bacc.Bacc("TRN2", target_bir_lowering=False, debug=False)

    def din(name, shape, dt=BF16):
        return nc.dram_tensor(name, list(shape), dt, kind="ExternalInput").ap()

    xT = din("xT", [128, 2, cap])          # x rows, feature-major tiled
    w1 = din("w1", [128, 2, S1])
    w2 = din("w2", [128, 8, S2])
    hw1 = din("hw1", [128, 8, H1])
    hw2 = din("hw2", [128, 4, H2])
    hw3 = din("hw3", [128, 2])
    # biases packed into one tensor: [b1(8) | b2(8) | hb1(4) | hb2(2) | hb3]
    cst = din("consts", [128, 23], F32)
    out = nc.dram_tensor("out", [1, cap], F32, kind="ExternalOutput").ap()

    n_chunks = N_CHUNKS
    chunk = cap // n_chunks
    assert chunk * n_chunks == cap and chunk % 32 == 0
    chunks = [(i * chunk, chunk) for i in range(n_chunks)]
    ALU = mybir.AluOpType

    with tile.TileContext(nc) as tc:
        with tc.tile_pool(name="weights", bufs=1) as wp, \
             tc.tile_pool(name="xin", bufs=n_chunks) as xp, \
             tc.tile_pool(name="acts", bufs=3) as ap, \
             tc.tile_pool(name="outs", bufs=2) as op, \
             tc.tile_pool(name="psum", bufs=6, space=MemorySpace.PSUM) as pp, \
             tc.tile_pool(name="psum_l", bufs=2, space=MemorySpace.PSUM) as plp:

            w1s = wp.tile([128, 2, S1], BF16, tag="w1s")
            w2s = wp.tile([128, 8, S2], BF16, tag="w2s")
            hw1s = wp.tile([128, 8, H1], BF16, tag="hw1s")
            hw2s = wp.tile([128, 4, H2], BF16, tag="hw2s")
            hw3s = wp.tile([128, 2], BF16, tag="hw3s")
            csts = wp.tile([128, 23], F32, tag="csts")
            b1s = csts[:, 0:8]
            b2s = csts[:, 8:16]
            hb1s = csts[:, 16:20]
            hb2s = csts[:, 20:22]
            hb3s = csts[:1, 22:23]

            # Two DMA rings in parallel: weights stream on SP (sync),
            # x chunks + consts on ACT (scalar), each in consumption order.
            for k in range(2):
                nc.sync.dma_start(out=w1s[:, k, :], in_=w1[:, k, :])
            xts = []
            for n0, N in chunks:
                xt = xp.tile([128, 2, chunk], BF16, tag="xt")
                if not xts:
                    # chunk 0 split per k so the first matmul is gated on
                    # half the data
                    for k in range(2):
                        nc.scalar.dma_start(out=xt[:, k, :N],
                                            in_=xT[:, k, n0:n0 + N])
                else:
                    nc.scalar.dma_start(out=xt[:, :, :N],
                                        in_=xT[:, :, n0:n0 + N])
                xts.append(xt)
                if len(xts) == 1:
                    nc.scalar.dma_start(out=csts[:], in_=cst[:])
            for k in range(8):
                nc.sync.dma_start(out=w2s[:, k, :], in_=w2[:, k, :])
            for k in range(8):
                nc.sync.dma_start(out=hw1s[:, k, :], in_=hw1[:, k, :])
            nc.sync.dma_start(out=hw2s[:], in_=hw2[:])
            nc.sync.dma_start(out=hw3s[:], in_=hw3[:])

            # PE warm-up: dependency-free matmuls fill the initial DMA-wait
            # window and release the HAM clock throttle (~3.4us of busy PE
            # needed for 1.2 -> 2.4 GHz) before the real matmuls arrive.
            if WARMUP_MMS:
                # Sized to span the gap between the PE preamble (~7.5us) and
                # the first weights landing (~11us): too short and HAM
                # re-throttles before the real matmuls, too long and the
                # FIFO delays them.
                wut = wp.tile([128, chunk], BF16, tag="wut")
                nc.vector.memset(wut[:], 0.0)
                wups = plp.tile([1, chunk], F32, tag="psl")
                for _ in range(WARMUP_MMS):
                    nc.tensor.matmul(wups[:1, :chunk], wut[:, 0:1],
                                     wut[:, :chunk], start=True, stop=True)

            # Bias+relu epilogue, alternating between ACT and DVE so neither
            # engine gates the PE during low-arithmetic layers.
            epi_n = [0]

            def epilogue(dst, src, bias_ap):
                if epi_n[0] % 2 == 0:
                    nc.scalar.activation(dst, src, AF.Relu, bias=bias_ap)
                else:
                    nc.vector.tensor_scalar(
                        dst, src, bias_ap, 0.0, ALU.add, ALU.max)
                epi_n[0] += 1

            for ci, (n0, N) in enumerate(chunks):
                xt = xts[ci]

                # L1: 256 -> 1024, relu
                h1 = ap.tile([128, 8, chunk], BF16, tag="h1")
                for m in range(8):
                    ps = pp.tile([128, chunk], F32, tag="ps")
                    for k in range(2):
                        nc.tensor.matmul(
                            ps[:, :N], w1s[:, k, m * 128:(m + 1) * 128],
                            xt[:, k, :N], start=(k == 0), stop=(k == 1))
                    epilogue(h1[:, m, :N], ps[:, :N], b1s[:, m:m + 1])

                # L2: 1024 -> 1024, relu
                h2 = ap.tile([128, 8, chunk], BF16, tag="h2")
                for m in range(8):
                    ps = pp.tile([128, chunk], F32, tag="ps")
                    for k in range(8):
                        nc.tensor.matmul(
                            ps[:, :N], w2s[:, k, m * 128:(m + 1) * 128],
                            h1[:, k, :N], start=(k == 0), stop=(k == 7))
                    epilogue(h2[:, m, :N], ps[:, :N], b2s[:, m:m + 1])

                # Head L1: 1024 -> 512, relu
                a1 = ap.tile([128, 4, chunk], BF16, tag="a1")
                for m in range(4):
                    ps = pp.tile([128, chunk], F32, tag="ps")
                    for k in range(8):
                        nc.tensor.matmul(
                            ps[:, :N], hw1s[:, k, m * 128:(m + 1) * 128],
                            h2[:, k, :N], start=(k == 0), stop=(k == 7))
                    epilogue(a1[:, m, :N], ps[:, :N], hb1s[:, m:m + 1])

                # Head L2: 512 -> 256, relu
                a2 = ap.tile([128, 2, chunk], BF16, tag="a2")
                for m in range(2):
                    ps = pp.tile([128, chunk], F32, tag="ps")
                    for k in range(4):
                        nc.tensor.matmul(
                            ps[:, :N], hw2s[:, k, m * 128:(m + 1) * 128],
                            a1[:, k, :N], start=(k == 0), stop=(k == 3))
                    epilogue(a2[:, m, :N], ps[:, :N], hb2s[:, m:m + 1])

                # Head L3: 256 -> 1, sigmoid
                psl = plp.tile([1, chunk], F32, tag="psl")
                for k in range(2):
                    nc.tensor.matmul(psl[:, :N], hw3s[:, k:k + 1],
                                     a2[:, k, :N],
                                     start=(k == 0), stop=(k == 1))
                ot = op.tile([1, chunk], F32, tag="ot")
                nc.scalar.activation(ot[:, :N], psl[:, :N], AF.Sigmoid,
                                     bias=hb3s[:1, :1])
                nc.sync.dma_start(out=out[:, n0:n0 + N], in_=ot[:, :N])

    nc.compile()
    return nc


def _get_nc(cap=CAP):
    if cap not in _nc_cache:
        _nc_cache[cap] = _build(cap)
    return _nc_cache[cap]


def _tile_k(w, ktiles):
    """[K, M] -> [128, ktiles, M] bf16 with K = ktiles*128, K idx = k*128+p."""
    k, m = w.shape
    assert k == ktiles * 128
    return np.ascontiguousarray(
        w.reshape(ktiles, 128, m).transpose(1, 0, 2).astype(NPBF16))


def _tile_b(b):
    """[M] -> [128, M/128] f32; column m holds bias for m-tile m."""
    m = b.shape[0]
    return np.ascontiguousarray(b.reshape(m // 128, 128).T.astype(np.float32))


def _make_in_maps(inputs):
    x = np.asarray(inputs["x"], dtype=np.float32)
    ff = np.asarray(inputs["feature_flags"]).astype(np.int64)
    idx = ff[:, 0] * 2 + ff[:, 1]

    W1 = np.asarray(inputs["W1"], np.float32)
    b1 = np.asarray(inputs["b1"], np.float32)
    W2 = np.asarray(inputs["W2"], np.float32)
    b2 = np.asarray(inputs["b2"], np.float32)
    HW1 = np.asarray(inputs["HW1"], np.float32)
    Hb1 = np.asarray(inputs["Hb1"], np.float32)
    HW2 = np.asarray(inputs["HW2"], np.float32)
    Hb2 = np.asarray(inputs["Hb2"], np.float32)
    HW3 = np.asarray(inputs["HW3"], np.float32)
    Hb3 = np.asarray(inputs["Hb3"], np.float32)

    # Row assignment: combo c -> cores 2c, 2c+1.
    row_sets = []
    for c in range(C):
        rows = np.nonzero(idx == c)[0]
        h = (len(rows) + 1) // 2
        row_sets.append(rows[:h])
        row_sets.append(rows[h:])
    max_shard = max(len(r) for r in row_sets)
    # cap = smallest multiple of 32*N_CHUNKS that fits every shard
    step = 32 * N_CHUNKS
    cap = max(CAP, -(-max_shard // step) * step)

    w1t = _tile_k(W1, 2)
    w2t = _tile_k(W2, 8)
    hw1t = [_tile_k(HW1[c], 8) for c in range(C)]
    hw2t = [_tile_k(HW2[c], 4) for c in range(C)]
    hw3t = [np.ascontiguousarray(
        HW3[c][:, 0].reshape(2, 128).T.astype(NPBF16)) for c in range(C)]
    cstt = []
    for c in range(C):
        cst = np.zeros((128, 23), np.float32)
        cst[:, 0:8] = _tile_b(b1)
        cst[:, 8:16] = _tile_b(b2)
        cst[:, 16:20] = _tile_b(Hb1[c])
        cst[:, 20:22] = _tile_b(Hb2[c])
        cst[:, 22] = np.float32(Hb3[c][0])
        cstt.append(cst)

    in_maps = []
    for d, rows in enumerate(row_sets):
        c = d // 2
        n = len(rows)
        xt = np.zeros((128, 2, cap), NPBF16)
        if n:
            xt[:, :, :n] = x[rows].T.reshape(2, 128, n).transpose(
                1, 0, 2).astype(NPBF16)
        in_maps.append({
            "xT": xt,
            "w1": w1t, "w2": w2t,
            "hw1": hw1t[c], "hw2": hw2t[c], "hw3": hw3t[c],
            "consts": cstt[c],
        })

    return in_maps, row_sets, cap


def kernel(**inputs):
    global _last_results
    in_maps, row_sets, cap = _make_in_maps(inputs)
    nc = _get_nc(cap)
    res = run_bass_kernel_spmd(nc, in_maps, core_ids=list(range(NCORES)))
    _last_results = res

    out = np.empty(B, np.float32)
    for d, rows in enumerate(row_sets):
        if len(rows):
            out[rows] = res.results[d]["out"][0, :len(rows)]
    return out



# revision 3
# speedup vs baseline: 1.5423x; 1.5423x over previous
"""Trainium2 Bass kernel for ConditionalNeuralNetwork (MoE-style routed MLP).

Strategy (expert-parallel over combos, data-parallel within a combo):
  - Host computes combo idx = 2*flags[:,0] + flags[:,1] per row, groups rows
    by combo, and splits each combo's rows across 2 of the 8 cores.
  - Each core runs a dense MLP 256 -> 1024 -> 1024 -> 512 -> 256 -> 1 on its
    rows with only ITS head's weights (relu between layers, sigmoid at end).
  - All matmuls except the final 256->1 run in fp8(e4m3) with
    perf_mode=DoubleRow (2 fp8 weights per PE cell = 2x MACs/cycle).
    Accumulation is fp32 in PSUM; epilogues (bias+relu) run on ACT/DVE and
    write fp8 activations directly.
  - Loop order is weight-stationary: for each (m-tile, k-pair) the stationary
    weights are loaded once and all row-chunks stream through, so the
    (expensive, non-overlapped in DoubleRow mode) LDWEIGHTS is amortized.
    Redundant LDWEIGHTS of the same weights are deleted post-build.
  - Epilogues are batched 2 chunks per instruction via 2-bank PSUM tiles.
  - Host scatters per-core outputs back to original row order.
"""

import os
import sys

import ml_dtypes
import numpy as np

for _p in ("/opt/trn_rl_repo", "/root/.axon_site/_ro/trn_rl_repo"):
    if os.path.isdir(_p) and _p not in sys.path:
        sys.path.append(_p)

import concourse.bacc as bacc
import concourse.bass as bass
import concourse.tile as tile
from concourse import mybir
from concourse.bass import MemorySpace
from concourse.bass_utils import run_bass_kernel_spmd

F32 = mybir.dt.float32
BF16 = mybir.dt.bfloat16
F8 = mybir.dt.float8e4
AF = mybir.ActivationFunctionType
DR = mybir.MatmulPerfMode.DoubleRow
NPBF16 = ml_dtypes.bfloat16
NPF8 = ml_dtypes.float8_e4m3  # TRN fp8e4: max +-240, RNE

B, D_IN = 16384, 256
S1, S2 = 1024, 1024
H1, H2 = 512, 256
C = 4
NCORES = 8
N_CHUNKS = 6
CAP = 2112  # rows per core, N_CHUNKS * CH; seed-0 max shard is 2080
CH = CAP // N_CHUNKS  # 352
WARMUP_MMS = 5

_nc_cache = {}
_last_results = None


def _dedup_ldweights(nc):
    """Remove back-to-back InstLdweights that reload identical weights.

    The rust add_instruction splits every matmul into LDWEIGHTS + MATMUL.
    With the weight-stationary loop order most loads are redundant; the PE
    keeps the stationary operand between matmuls. Any waits on a removed
    LDWEIGHTS are merged into the instruction that followed it.
    """
    removed = kept = 0
    for f in nc.m.functions:
        for blk in f.blocks:
            insts = list(blk.instructions)
            new = []
            last_key = None
            pending_waits = []
            for inst in insts:
                nm = type(inst).__name__
                if nm == "InstLdweights":
                    key = (repr(inst.ins[0]), inst.perf_mode,
                           inst.tile_position, inst.is_transpose)
                    si = inst.sync_info
                    has_upd = bool(si is not None and si.on_update)
                    if key == last_key and not has_upd:
                        if si is not None and si.on_wait:
                            pending_waits.extend(si.on_wait)
                        removed += 1
                        continue
                    last_key = key
                    kept += 1
                elif nm == "InstMatmult":
                    if pending_waits:
                        si = inst.sync_info
                        if si is None:
                            inst.sync_info = mybir.SyncInfo(
                                on_wait=list(pending_waits), on_update=[])
                        else:
                            si.on_wait = list(si.on_wait) + pending_waits
                        pending_waits = []
                # Other instruction kinds run on non-PE engines (or are
                # semaphore ops) and do not disturb the PE weight array, so
                # the cached key stays valid across them.
                new.append(inst)
            assert not pending_waits
            blk.instructions[:] = new
    return removed, kept


def _build(cap=CAP):
    """Build the single-core MLP program (SPMD across 8 cores)."""
    nc = bacc.Bacc("TRN2", target_bir_lowering=False, debug=False)

    def din(name, shape, dt=F8):
        return nc.dram_tensor(name, list(shape), dt, kind="ExternalInput").ap()

    n_chunks = N_CHUNKS
    ch = cap // n_chunks
    assert ch * n_chunks == cap and ch % 32 == 0 and ch <= 512

    xT = din("xT", [128, 2, n_chunks, ch])   # x rows, feature-major tiled
    w1 = din("w1", [128, 2, S1])
    w2 = din("w2", [128, 8, S2])
    hw1 = din("hw1", [128, 8, H1])
    hw2 = din("hw2", [128, 4, H2])
    hw3 = din("hw3", [128, 2, 16])           # w3 in col 0, zero-padded
    # biases packed into one tensor: [b1(8) | b2(8) | hb1(4) | hb2(2) | hb3]
    cst = din("consts", [128, 23], F32)
    out = nc.dram_tensor("out", [1, cap], F32, kind="ExternalOutput").ap()

    ALU = mybir.AluOpType

    with tile.TileContext(nc) as tc:
        with tc.tile_pool(name="weights", bufs=1) as wp, \
             tc.tile_pool(name="acts", bufs=1) as ap_, \
             tc.tile_pool(name="outs", bufs=2) as op, \
             tc.tile_pool(name="psum", bufs=4, space=MemorySpace.PSUM) as pp:

            w1s = wp.tile([128, 2, S1], F8, tag="w1s")
            w2s = wp.tile([128, 8, S2], F8, tag="w2s")
            hw1s = wp.tile([128, 8, H1], F8, tag="hw1s")
            hw2s = wp.tile([128, 4, H2], F8, tag="hw2s")
            hw3s = wp.tile([128, 2, 16], F8, tag="hw3s")
            csts = wp.tile([128, 23], F32, tag="csts")
            b1s = csts[:, 0:8]
            b2s = csts[:, 8:16]
            hb1s = csts[:, 16:20]
            hb2s = csts[:, 20:22]
            hb3s = csts[:1, 22:23]

            # activations stay resident for all chunks (weight-stationary)
            xts = ap_.tile([128, 2, n_chunks, ch], F8, tag="xts")
            h1s = ap_.tile([128, 8, n_chunks, ch], F8, tag="h1s")
            h2s = ap_.tile([128, 8, n_chunks, ch], F8, tag="h2s")
            a1s = ap_.tile([128, 4, n_chunks, ch], F8, tag="a1s")
            a2s = ap_.tile([128, 2, n_chunks, ch], F8, tag="a2s")

            # DMA: weights on SP (sync) queue, x + consts on ACT queue,
            # both in consumption order.
            for k in range(2):
                nc.sync.dma_start(out=w1s[:, k, :], in_=w1[:, k, :])
            for c in range(n_chunks):
                nc.scalar.dma_start(out=xts[:, :, c, :], in_=xT[:, :, c, :])
                if c == 0:
                    nc.scalar.dma_start(out=csts[:], in_=cst[:])
            for k in range(8):
                nc.sync.dma_start(out=w2s[:, k, :], in_=w2[:, k, :])
            for k in range(8):
                nc.sync.dma_start(out=hw1s[:, k, :], in_=hw1[:, k, :])
            nc.sync.dma_start(out=hw2s[:], in_=hw2[:])
            nc.sync.dma_start(out=hw3s[:], in_=hw3[:])

            # PE warm-up: dependency-free matmuls fill the initial DMA-wait
            # window and release the HAM clock throttle before real matmuls.
            if WARMUP_MMS:
                wut = wp.tile([128, ch], BF16, tag="wut")
                nc.vector.memset(wut[:], 0.0)
                wups = pp.tile([128, 2, 512], F32, tag="ps")
                for _ in range(WARMUP_MMS):
                    nc.tensor.matmul(wups[:1, 0, :ch], wut[:, 0:1],
                                     wut[:, :ch], start=True, stop=True)

            # Bias+relu epilogue, alternating ACT / DVE.
            epi_n = [0]

            def epilogue(dst, src, bias_ap, func=AF.Relu):
                if epi_n[0] % 2 == 0:
                    nc.scalar.activation(dst, src, func, bias=bias_ap)
                else:
                    nc.vector.tensor_scalar(
                        dst, src, bias_ap, 0.0, ALU.add, ALU.max)
                epi_n[0] += 1

            def layer(src, ktiles, wt, nm, bias, dst):
                """dst[:,m,:,:] = relu(sum_k wt[:,k,m]T @ src[:,k,c,:] + b)"""
                npair = ktiles // 2
                for m in range(nm):
                    t01 = pp.tile([128, 2, 512], F32, tag="ps")
                    t23 = pp.tile([128, 2, 512], F32, tag="ps")
                    t45 = pp.tile([128, 2, 512], F32, tag="ps")
                    tt = (t01, t23, t45)
                    for kp in range(npair):
                        lhs = wt[:, 2 * kp:2 * kp + 2, m * 128:(m + 1) * 128]
                        for c in range(n_chunks):
                            nc.tensor.matmul(
                                tt[c // 2][:, c % 2, :ch], lhs,
                                src[:, 2 * kp:2 * kp + 2, c, :],
                                start=(kp == 0), stop=(kp == npair - 1),
                                perf_mode=DR)
                    for j in range(3):
                        epilogue(dst[:, m, 2 * j:2 * j + 2, :],
                                 tt[j][:, :, :ch], bias[:, m:m + 1])

            layer(xts, 2, w1s, 8, b1s, h1s)      # L1: 256 -> 1024
            layer(h1s, 8, w2s, 8, b2s, h2s)      # L2: 1024 -> 1024
            layer(h2s, 8, hw1s, 4, hb1s, a1s)    # HL1: 1024 -> 512
            layer(a1s, 4, hw2s, 2, hb2s, a2s)    # HL2: 512 -> 256

            # HL3: 256 -> 1, sigmoid. DoubleRow with M=1; 2 chunks per psum
            # tile (partition 0 of each bank), one sigmoid per 2 chunks.
            for j in range(n_chunks // 2):
                psl = pp.tile([128, 2, 512], F32, tag="ps")
                for c2 in range(2):
                    c = 2 * j + c2
                    nc.tensor.matmul(psl[0:1, c2, :ch], hw3s[:, :, 0:1],
                                     a2s[:, :, c, :], start=True, stop=True,
                                     perf_mode=DR)
                ot = op.tile([1, 2, ch], F32, tag="ot")
                nc.scalar.activation(ot[:, :, :], psl[0:1, :, :ch],
                                     AF.Sigmoid, bias=hb3s[:1, :1])
                nc.sync.dma_start(out=out[:, 2 * j * ch:(2 * j + 2) * ch],
                                  in_=ot[:, :, :])

    removed, kept = _dedup_ldweights(nc)
    nc.compile()
    return nc


def _get_nc(cap=CAP):
    if cap not in _nc_cache:
        _nc_cache[cap] = _build(cap)
    return _nc_cache[cap]


def _q8(v):
    return np.clip(v, -240.0, 240.0).astype(NPF8)


def _tile_k(w, ktiles):
    """[K, M] -> [128, ktiles, M] fp8 with K = ktiles*128, K idx = k*128+p."""
    k, m = w.shape
    assert k == ktiles * 128
    return np.ascontiguousarray(
        _q8(w.reshape(ktiles, 128, m).transpose(1, 0, 2)))


def _tile_b(b):
    """[M] -> [128, M/128] f32; column m holds bias for m-tile m."""
    m = b.shape[0]
    return np.ascontiguousarray(b.reshape(m // 128, 128).T.astype(np.float32))


def _make_in_maps(inputs):
    x = np.asarray(inputs["x"], dtype=np.float32)
    ff = np.asarray(inputs["feature_flags"]).astype(np.int64)
    idx = ff[:, 0] * 2 + ff[:, 1]

    W1 = np.asarray(inputs["W1"], np.float32)
    b1 = np.asarray(inputs["b1"], np.float32)
    W2 = np.asarray(inputs["W2"], np.float32)
    b2 = np.asarray(inputs["b2"], np.float32)
    HW1 = np.asarray(inputs["HW1"], np.float32)
    Hb1 = np.asarray(inputs["Hb1"], np.float32)
    HW2 = np.asarray(inputs["HW2"], np.float32)
    Hb2 = np.asarray(inputs["Hb2"], np.float32)
    HW3 = np.asarray(inputs["HW3"], np.float32)
    Hb3 = np.asarray(inputs["Hb3"], np.float32)

    # Row assignment: combo c -> cores 2c, 2c+1.
    row_sets = []
    for c in range(C):
        rows = np.nonzero(idx == c)[0]
        h = (len(rows) + 1) // 2
        row_sets.append(rows[:h])
        row_sets.append(rows[h:])
    max_shard = max(len(r) for r in row_sets)
    # cap = smallest multiple of 32*N_CHUNKS that fits every shard
    step = 32 * N_CHUNKS
    cap = max(CAP, -(-max_shard // step) * step)

    w1t = _tile_k(W1, 2)
    w2t = _tile_k(W2, 8)
    hw1t = [_tile_k(HW1[c], 8) for c in range(C)]
    hw2t = [_tile_k(HW2[c], 4) for c in range(C)]
    hw3t = []
    for c in range(C):
        t = np.zeros((128, 2, 16), NPF8)
        t[:, :, 0] = _q8(HW3[c][:, 0].reshape(2, 128).T)
        hw3t.append(t)
    cstt = []
    for c in range(C):
        cst = np.zeros((128, 23), np.float32)
        cst[:, 0:8] = _tile_b(b1)
        cst[:, 8:16] = _tile_b(b2)
        cst[:, 16:20] = _tile_b(Hb1[c])
        cst[:, 20:22] = _tile_b(Hb2[c])
        cst[:, 22] = np.float32(Hb3[c][0])
        cstt.append(cst)

    n_chunks = N_CHUNKS
    in_maps = []
    for d, rows in enumerate(row_sets):
        c = d // 2
        n = len(rows)
        ch = cap // n_chunks
        xt = np.zeros((128, 2, cap), NPF8)
        if n:
            xt[:, :, :n] = _q8(x[rows].T.reshape(2, 128, n).transpose(
                1, 0, 2))
        xt = np.ascontiguousarray(
            xt.reshape(128, 2, n_chunks, ch))
        in_maps.append({
            "xT": xt,
            "w1": w1t, "w2": w2t,
            "hw1": hw1t[c], "hw2": hw2t[c], "hw3": hw3t[c],
            "consts": cstt[c],
        })

    return in_maps, row_sets, cap


def kernel(**inputs):
    global _last_results
    in_maps, row_sets, cap = _make_in_maps(inputs)
    nc = _get_nc(cap)
    res = run_bass_kernel_spmd(nc, in_maps, core_ids=list(range(NCORES)))
    _last_results = res

    out = np.empty(B, np.float32)
    for d, rows in enumerate(row_sets):
        if len(rows):
            out[rows] = res.results[d]["out"][0, :len(rows)]
    return out


# revision 9
# speedup vs baseline: 1.5679x; 1.0166x over previous
"""Trainium2 Bass kernel for ConditionalNeuralNetwork (MoE-style routed MLP).

Strategy (expert-parallel over combos, data-parallel within a combo):
  - Host computes combo idx = 2*flags[:,0] + flags[:,1] per row, groups rows
    by combo, and splits each combo's rows across 2 of the 8 cores.
  - Each core runs a dense MLP 256 -> 1024 -> 1024 -> 512 -> 256 -> 1 on its
    rows with only ITS head's weights (relu between layers, sigmoid at end).
  - All matmuls except the final 256->1 run in fp8(e4m3) with
    perf_mode=DoubleRow (2 fp8 weights per PE cell = 2x MACs/cycle).
    Accumulation is fp32 in PSUM; epilogues (bias+relu) run on ACT/DVE and
    write fp8 activations directly.
  - Loop order is weight-stationary: for each (m-tile, k-pair) the stationary
    weights are loaded once and all row-chunks stream through, so the
    (expensive, non-overlapped in DoubleRow mode) LDWEIGHTS is amortized.
    Redundant LDWEIGHTS of the same weights are deleted post-build.
  - Epilogues are batched 2 chunks per instruction via 2-bank PSUM tiles.
  - Host scatters per-core outputs back to original row order.
"""

import os
import sys

import ml_dtypes
import numpy as np

for _p in ("/opt/trn_rl_repo", "/root/.axon_site/_ro/trn_rl_repo"):
    if os.path.isdir(_p) and _p not in sys.path:
        sys.path.append(_p)

import concourse.bacc as bacc
import concourse.bass as bass
import concourse.tile as tile
from concourse import mybir
from concourse.bass import MemorySpace
from concourse.bass_utils import run_bass_kernel_spmd

F32 = mybir.dt.float32
BF16 = mybir.dt.bfloat16
F8 = mybir.dt.float8e4
AF = mybir.ActivationFunctionType
DR = mybir.MatmulPerfMode.DoubleRow
NPBF16 = ml_dtypes.bfloat16
NPF8 = ml_dtypes.float8_e4m3  # TRN fp8e4: max +-240, RNE

B, D_IN = 16384, 256
S1, S2 = 1024, 1024
H1, H2 = 512, 256
C = 4
NCORES = 8
N_CHUNKS = 6
CAP = 2112  # rows per core, N_CHUNKS * CH; seed-0 max shard is 2080
CH = CAP // N_CHUNKS  # 352
WARMUP_MMS = 5

_nc_cache = {}
_last_results = None


def _dedup_ldweights(nc):
    """Remove back-to-back InstLdweights that reload identical weights.

    The rust add_instruction splits every matmul into LDWEIGHTS + MATMUL.
    With the weight-stationary loop order most loads are redundant; the PE
    keeps the stationary operand between matmuls. Any waits on a removed
    LDWEIGHTS are merged into the instruction that followed it.
    """
    removed = kept = 0
    for f in nc.m.functions:
        for blk in f.blocks:
            insts = list(blk.instructions)
            new = []
            last_key = None
            pending_waits = []
            for inst in insts:
                nm = type(inst).__name__
                if nm == "InstLdweights":
                    key = (repr(inst.ins[0]), inst.perf_mode,
                           inst.tile_position, inst.is_transpose)
                    si = inst.sync_info
                    has_upd = bool(si is not None and si.on_update)
                    if key == last_key and not has_upd:
                        if si is not None and si.on_wait:
                            pending_waits.extend(si.on_wait)
                        removed += 1
                        continue
                    last_key = key
                    kept += 1
                elif nm == "InstMatmult":
                    if pending_waits:
                        si = inst.sync_info
                        if si is None:
                            inst.sync_info = mybir.SyncInfo(
                                on_wait=list(pending_waits), on_update=[])
                        else:
                            si.on_wait = list(si.on_wait) + pending_waits
                        pending_waits = []
                # Other instruction kinds run on non-PE engines (or are
                # semaphore ops) and do not disturb the PE weight array, so
                # the cached key stays valid across them.
                new.append(inst)
            assert not pending_waits
            blk.instructions[:] = new
    return removed, kept


def _build(cap=CAP):
    """Build the single-core MLP program (SPMD across 8 cores)."""
    nc = bacc.Bacc("TRN2", target_bir_lowering=False, debug=False)

    def din(name, shape, dt=F8):
        return nc.dram_tensor(name, list(shape), dt, kind="ExternalInput").ap()

    n_chunks = N_CHUNKS
    ch = cap // n_chunks
    assert ch * n_chunks == cap and ch % 32 == 0 and ch <= 512

    xT = din("xT", [128, n_chunks, 2, ch])   # x rows, chunk-major tiled
    w1 = din("w1", [128, 2, S1])
    w2 = din("w2", [128, 8, S2])
    hw1 = din("hw1", [128, 8, H1])
    hw2 = din("hw2", [128, 4, H2])
    hw3 = din("hw3", [128, 2, 16])           # w3 in col 0, zero-padded
    # biases packed into one tensor: [b1(8) | b2(8) | hb1(4) | hb2(2) | hb3]
    cst = din("consts", [128, 23], F32)
    out = nc.dram_tensor("out", [1, cap], F32, kind="ExternalOutput").ap()

    ALU = mybir.AluOpType

    with tile.TileContext(nc) as tc:
        with tc.tile_pool(name="weights", bufs=1) as wp, \
             tc.tile_pool(name="acts", bufs=1) as ap_, \
             tc.tile_pool(name="outs", bufs=2) as op, \
             tc.tile_pool(name="psum", bufs=4, space=MemorySpace.PSUM) as pp:

            w1s = wp.tile([128, 2, S1], F8, tag="w1s")
            w2s = wp.tile([128, 8, S2], F8, tag="w2s")
            hw1s = wp.tile([128, 8, H1], F8, tag="hw1s")
            hw2s = wp.tile([128, 4, H2], F8, tag="hw2s")
            hw3s = wp.tile([128, 2, 16], F8, tag="hw3s")
            csts = wp.tile([128, 23], F32, tag="csts")
            b1s = csts[:, 0:8]
            b2s = csts[:, 8:16]
            hb1s = csts[:, 16:20]
            hb2s = csts[:, 20:22]
            hb3s = csts[:1, 22:23]

            # activations stay resident for all chunks (weight-stationary)
            xts = ap_.tile([128, n_chunks, 2, ch], F8, tag="xts")
            h1s = ap_.tile([128, 8, n_chunks, ch], F8, tag="h1s")
            h2s = ap_.tile([128, 8, n_chunks, ch], F8, tag="h2s")
            a1s = ap_.tile([128, 4, n_chunks, ch], F8, tag="a1s")
            a2s = ap_.tile([128, 2, n_chunks, ch], F8, tag="a2s")

            # DMA: weights on SP (sync) queue; x split over 3 queues with
            # chunk-contiguous lines (1408B per partition per piece).
            for k in range(2):
                nc.sync.dma_start(out=w1s[:, k, :], in_=w1[:, k, :])
            xq = (nc.scalar, nc.gpsimd, nc.sync)
            for j in range(3):
                xq[j].dma_start(out=xts[:, 2 * j:2 * j + 2, :, :],
                                in_=xT[:, 2 * j:2 * j + 2, :, :])
            nc.scalar.dma_start(out=csts[:], in_=cst[:])
            for k in range(8):
                nc.sync.dma_start(out=w2s[:, k, :], in_=w2[:, k, :])
            for k in range(8):
                nc.sync.dma_start(out=hw1s[:, k, :], in_=hw1[:, k, :])
            nc.sync.dma_start(out=hw2s[:], in_=hw2[:])
            nc.sync.dma_start(out=hw3s[:], in_=hw3[:])

            # PE warm-up: dependency-free matmuls fill the initial DMA-wait
            # window and release the HAM clock throttle before real matmuls.
            if WARMUP_MMS:
                wut = wp.tile([128, ch], BF16, tag="wut")
                nc.vector.memset(wut[:], 0.0)
                wups = pp.tile([128, 2, 512], F32, tag="ps")
                for _ in range(WARMUP_MMS):
                    nc.tensor.matmul(wups[:1, 0, :ch], wut[:, 0:1],
                                     wut[:, :ch], start=True, stop=True)

            # Bias+relu epilogue, alternating ACT / DVE.
            epi_n = [0]

            def epilogue(dst, src, bias_ap, func=AF.Relu):
                if epi_n[0] % 2 == 0:
                    nc.scalar.activation(dst, src, func, bias=bias_ap)
                else:
                    nc.vector.tensor_scalar(
                        dst, src, bias_ap, 0.0, ALU.add, ALU.max)
                epi_n[0] += 1

            def layer(rhs, ktiles, wt, nm, bias, dst):
                """dst[:,m,:,:] = relu(sum_k wt[:,k,m]T @ rhs(kp,c) + b)"""
                npair = ktiles // 2
                for m in range(nm):
                    t01 = pp.tile([128, 2, 512], F32, tag="ps")
                    t23 = pp.tile([128, 2, 512], F32, tag="ps")
                    t45 = pp.tile([128, 2, 512], F32, tag="ps")
                    tt = (t01, t23, t45)
                    for kp in range(npair):
                        lhs = wt[:, 2 * kp:2 * kp + 2, m * 128:(m + 1) * 128]
                        for c in range(n_chunks):
                            nc.tensor.matmul(
                                tt[c // 2][:, c % 2, :ch], lhs, rhs(kp, c),
                                start=(kp == 0), stop=(kp == npair - 1),
                                perf_mode=DR)
                    for j in range(3):
                        epilogue(dst[:, m, 2 * j:2 * j + 2, :],
                                 tt[j][:, :, :ch], bias[:, m:m + 1])

            def hslice(t):
                return lambda kp, c: t[:, 2 * kp:2 * kp + 2, c, :]

            layer(lambda kp, c: xts[:, c, :, :], 2, w1s, 8, b1s, h1s)
            layer(hslice(h1s), 8, w2s, 8, b2s, h2s)      # L2: 1024 -> 1024
            layer(hslice(h2s), 8, hw1s, 4, hb1s, a1s)    # HL1: 1024 -> 512
            layer(hslice(a1s), 4, hw2s, 2, hb2s, a2s)    # HL2: 512 -> 256

            # HL3: 256 -> 1, sigmoid. DoubleRow with M=1; 2 chunks per psum
            # tile (partition 0 of each bank), one sigmoid per 2 chunks.
            for j in range(n_chunks // 2):
                psl = pp.tile([128, 2, 512], F32, tag="ps")
                for c2 in range(2):
                    c = 2 * j + c2
                    nc.tensor.matmul(psl[0:1, c2, :ch], hw3s[:, :, 0:1],
                                     a2s[:, :, c, :], start=True, stop=True,
                                     perf_mode=DR)
                ot = op.tile([1, 2, ch], F32, tag="ot")
                nc.scalar.activation(ot[:, :, :], psl[0:1, :, :ch],
                                     AF.Sigmoid, bias=hb3s[:1, :1])
                nc.sync.dma_start(out=out[:, 2 * j * ch:(2 * j + 2) * ch],
                                  in_=ot[:, :, :])

    removed, kept = _dedup_ldweights(nc)
    nc.compile()
    return nc


def _get_nc(cap=CAP):
    if cap not in _nc_cache:
        _nc_cache[cap] = _build(cap)
    return _nc_cache[cap]


def _q8(v):
    return np.clip(v, -240.0, 240.0).astype(NPF8)


def _tile_k(w, ktiles):
    """[K, M] -> [128, ktiles, M] fp8 with K = ktiles*128, K idx = k*128+p."""
    k, m = w.shape
    assert k == ktiles * 128
    return np.ascontiguousarray(
        _q8(w.reshape(ktiles, 128, m).transpose(1, 0, 2)))


def _tile_b(b):
    """[M] -> [128, M/128] f32; column m holds bias for m-tile m."""
    m = b.shape[0]
    return np.ascontiguousarray(b.reshape(m // 128, 128).T.astype(np.float32))


def _make_in_maps(inputs):
    x = np.asarray(inputs["x"], dtype=np.float32)
    ff = np.asarray(inputs["feature_flags"]).astype(np.int64)
    idx = ff[:, 0] * 2 + ff[:, 1]

    W1 = np.asarray(inputs["W1"], np.float32)
    b1 = np.asarray(inputs["b1"], np.float32)
    W2 = np.asarray(inputs["W2"], np.float32)
    b2 = np.asarray(inputs["b2"], np.float32)
    HW1 = np.asarray(inputs["HW1"], np.float32)
    Hb1 = np.asarray(inputs["Hb1"], np.float32)
    HW2 = np.asarray(inputs["HW2"], np.float32)
    Hb2 = np.asarray(inputs["Hb2"], np.float32)
    HW3 = np.asarray(inputs["HW3"], np.float32)
    Hb3 = np.asarray(inputs["Hb3"], np.float32)

    # Row assignment: combo c -> cores 2c, 2c+1.
    row_sets = []
    for c in range(C):
        rows = np.nonzero(idx == c)[0]
        h = (len(rows) + 1) // 2
        row_sets.append(rows[:h])
        row_sets.append(rows[h:])
    max_shard = max(len(r) for r in row_sets)
    # cap = smallest multiple of 32*N_CHUNKS that fits every shard
    step = 32 * N_CHUNKS
    cap = max(CAP, -(-max_shard // step) * step)

    w1t = _tile_k(W1, 2)
    w2t = _tile_k(W2, 8)
    hw1t = [_tile_k(HW1[c], 8) for c in range(C)]
    hw2t = [_tile_k(HW2[c], 4) for c in range(C)]
    hw3t = []
    for c in range(C):
        t = np.zeros((128, 2, 16), NPF8)
        t[:, :, 0] = _q8(HW3[c][:, 0].reshape(2, 128).T)
        hw3t.append(t)
    cstt = []
    for c in range(C):
        cst = np.zeros((128, 23), np.float32)
        cst[:, 0:8] = _tile_b(b1)
        cst[:, 8:16] = _tile_b(b2)
        cst[:, 16:20] = _tile_b(Hb1[c])
        cst[:, 20:22] = _tile_b(Hb2[c])
        cst[:, 22] = np.float32(Hb3[c][0])
        cstt.append(cst)

    n_chunks = N_CHUNKS
    in_maps = []
    for d, rows in enumerate(row_sets):
        c = d // 2
        n = len(rows)
        ch = cap // n_chunks
        xt = np.zeros((128, 2, cap), NPF8)
        if n:
            xt[:, :, :n] = _q8(x[rows].T.reshape(2, 128, n).transpose(
                1, 0, 2))
        # [128, 2, cap] -> [128, n_chunks, 2, ch] (chunk-major)
        xt = np.ascontiguousarray(
            xt.reshape(128, 2, n_chunks, ch).transpose(0, 2, 1, 3))
        in_maps.append({
            "xT": xt,
            "w1": w1t, "w2": w2t,
            "hw1": hw1t[c], "hw2": hw2t[c], "hw3": hw3t[c],
            "consts": cstt[c],
        })

    return in_maps, row_sets, cap


def kernel(**inputs):
    global _last_results
    in_maps, row_sets, cap = _make_in_maps(inputs)
    nc = _get_nc(cap)
    res = run_bass_kernel_spmd(nc, in_maps, core_ids=list(range(NCORES)))
    _last_results = res

    out = np.empty(B, np.float32)
    for d, rows in enumerate(row_sets):
        if len(rows):
            out[rows] = res.results[d]["out"][0, :len(rows)]
    return out


# revision 20
# speedup vs baseline: 1.6138x; 1.0292x over previous
"""Trainium2 Bass kernel for ConditionalNeuralNetwork (MoE-style routed MLP).

Strategy (expert-parallel over combos, data-parallel within a combo):
  - Host computes combo idx = 2*flags[:,0] + flags[:,1] per row, groups rows
    by combo, and splits each combo's rows across 2 of the 8 cores.
  - Each core runs a dense MLP 256 -> 1024 -> 1024 -> 512 -> 256 -> 1 on its
    rows with only ITS head's weights (relu between layers, sigmoid at end).
  - All matmuls except the final 256->1 run in fp8(e4m3) with
    perf_mode=DoubleRow (2 fp8 weights per PE cell = 2x MACs/cycle).
    Accumulation is fp32 in PSUM; epilogues (bias+relu) run on ACT/DVE and
    write fp8 activations directly.
  - Loop order is weight-stationary: for each (m-tile, k-pair) the stationary
    weights are loaded once and all row-chunks stream through, so the
    (expensive, non-overlapped in DoubleRow mode) LDWEIGHTS is amortized.
    Redundant LDWEIGHTS of the same weights are deleted post-build.
  - Epilogues are batched 2 chunks per instruction via 2-bank PSUM tiles.
  - Host scatters per-core outputs back to original row order.
"""

import os
import sys

import ml_dtypes
import numpy as np

for _p in ("/opt/trn_rl_repo", "/root/.axon_site/_ro/trn_rl_repo"):
    if os.path.isdir(_p) and _p not in sys.path:
        sys.path.append(_p)

import concourse.bacc as bacc
import concourse.bass as bass
import concourse.tile as tile
from concourse import mybir
from concourse.bass import MemorySpace
from concourse.bass_utils import run_bass_kernel_spmd

F32 = mybir.dt.float32
BF16 = mybir.dt.bfloat16
F8 = mybir.dt.float8e4
AF = mybir.ActivationFunctionType
DR = mybir.MatmulPerfMode.DoubleRow
SWI = mybir.MatmulPerfMode.DoubleRowSwInterleave
NPBF16 = ml_dtypes.bfloat16
NPF8 = ml_dtypes.float8_e4m3  # TRN fp8e4: max +-240, RNE

USE_SWI = os.environ.get("K_SWI", "1") == "1"

B, D_IN = 16384, 256
S1, S2 = 1024, 1024
H1, H2 = 512, 256
C = 4
NCORES = 8
N_CHUNKS = 6
CAP = 2112  # rows per core, N_CHUNKS * CH; seed-0 max shard is 2080
CH = CAP // N_CHUNKS  # 352
WARMUP_MMS = 5

_nc_cache = {}
_last_results = None


def _dedup_ldweights(nc):
    """Remove back-to-back InstLdweights that reload identical weights.

    The rust add_instruction splits every matmul into LDWEIGHTS + MATMUL.
    With the weight-stationary loop order most loads are redundant; the PE
    keeps the stationary operand between matmuls. Any waits on a removed
    LDWEIGHTS are merged into the instruction that followed it.
    """
    removed = kept = 0
    for f in nc.m.functions:
        for blk in f.blocks:
            insts = list(blk.instructions)
            new = []
            last_key = None
            pending_waits = []
            for inst in insts:
                nm = type(inst).__name__
                if nm == "InstLdweights":
                    key = (repr(inst.ins[0]), inst.perf_mode,
                           inst.tile_position, inst.is_transpose)
                    si = inst.sync_info
                    has_upd = bool(si is not None and si.on_update)
                    if key == last_key and not has_upd:
                        if si is not None and si.on_wait:
                            pending_waits.extend(si.on_wait)
                        removed += 1
                        continue
                    last_key = key
                    kept += 1
                elif nm == "InstMatmult":
                    if pending_waits:
                        si = inst.sync_info
                        if si is None:
                            inst.sync_info = mybir.SyncInfo(
                                on_wait=list(pending_waits), on_update=[])
                        else:
                            si.on_wait = list(si.on_wait) + pending_waits
                        pending_waits = []
                # Other instruction kinds run on non-PE engines (or are
                # semaphore ops) and do not disturb the PE weight array, so
                # the cached key stays valid across them.
                new.append(inst)
            assert not pending_waits
            blk.instructions[:] = new
    return removed, kept


def _build(cap=CAP):
    """Build the single-core MLP program (SPMD across 8 cores)."""
    nc = bacc.Bacc("TRN2", target_bir_lowering=False, debug=False)

    def din(name, shape, dt=F8):
        return nc.dram_tensor(name, list(shape), dt, kind="ExternalInput").ap()

    n_chunks = N_CHUNKS
    ch = cap // n_chunks
    assert ch * n_chunks == cap and ch % 32 == 0 and ch <= 512

    xT = din("xT", [128, n_chunks, 2, ch])   # x rows, chunk-major tiled
    if USE_SWI:
        # weights pre-interleaved per (k-pair, m-tile): [128, kp, m, 256]
        w1 = din("w1", [128, 1, 8, 2, 128])
        w2 = din("w2", [128, 4, 8, 2, 128])
        hw1 = din("hw1", [128, 4, 4, 2, 128])
        hw2 = din("hw2", [128, 2, 2, 2, 128])
        hw3 = din("hw3", [128, 2, 16])       # w3 in col 0, zero-padded
    else:
        w1 = din("w1", [128, 2, S1])
        w2 = din("w2", [128, 8, S2])
        hw1 = din("hw1", [128, 8, H1])
        hw2 = din("hw2", [128, 4, H2])
        hw3 = din("hw3", [128, 2, 16])       # w3 in col 0, zero-padded
    # biases packed into one tensor: [b1(8) | b2(8) | hb1(4) | hb2(2) | hb3]
    cst = din("consts", [128, 23], F32)
    out = nc.dram_tensor("out", [1, cap], F32, kind="ExternalOutput").ap()

    ALU = mybir.AluOpType

    with tile.TileContext(nc) as tc:
        with tc.tile_pool(name="weights", bufs=1) as wp, \
             tc.tile_pool(name="acts", bufs=1) as ap_, \
             tc.tile_pool(name="outs", bufs=2) as op, \
             tc.tile_pool(name="psum", bufs=4, space=MemorySpace.PSUM) as pp:

            if USE_SWI:
                w1s = wp.tile([128, 1, 8, 2, 128], F8, tag="w1s")
                w2s = wp.tile([128, 4, 8, 2, 128], F8, tag="w2s")
                hw1s = wp.tile([128, 4, 4, 2, 128], F8, tag="hw1s")
                hw2s = wp.tile([128, 2, 2, 2, 128], F8, tag="hw2s")
                hw3s = wp.tile([128, 2, 16], F8, tag="hw3s")
            else:
                w1s = wp.tile([128, 2, S1], F8, tag="w1s")
                w2s = wp.tile([128, 8, S2], F8, tag="w2s")
                hw1s = wp.tile([128, 8, H1], F8, tag="hw1s")
                hw2s = wp.tile([128, 4, H2], F8, tag="hw2s")
                hw3s = wp.tile([128, 2, 16], F8, tag="hw3s")
            csts = wp.tile([128, 23], F32, tag="csts")
            b1s = csts[:, 0:8]
            b2s = csts[:, 8:16]
            hb1s = csts[:, 16:20]
            hb2s = csts[:, 20:22]
            hb3s = csts[:1, 22:23]

            # activations stay resident for all chunks (weight-stationary)
            xts = ap_.tile([128, n_chunks, 2, ch], F8, tag="xts")
            h1s = ap_.tile([128, 8, n_chunks, ch], F8, tag="h1s")
            h2s = ap_.tile([128, 8, n_chunks, ch], F8, tag="h2s")
            a1s = ap_.tile([128, 4, n_chunks, ch], F8, tag="a1s")
            a2s = ap_.tile([128, 2, n_chunks, ch], F8, tag="a2s")

            # DMA: weights on SP (sync) queue; x split over 3 queues with
            # chunk-contiguous lines (1408B per partition per piece).
            if USE_SWI:
                nc.sync.dma_start(out=w1s[:], in_=w1[:])
            else:
                for k in range(2):
                    nc.sync.dma_start(out=w1s[:, k, :], in_=w1[:, k, :])
            xq = (nc.scalar, nc.gpsimd, nc.sync)
            for j in range(3):
                xq[j].dma_start(out=xts[:, 2 * j:2 * j + 2, :, :],
                                in_=xT[:, 2 * j:2 * j + 2, :, :])
            nc.scalar.dma_start(out=csts[:], in_=cst[:])
            if USE_SWI:
                for kp in range(4):
                    nc.sync.dma_start(out=w2s[:, kp], in_=w2[:, kp])
                for kp in range(4):
                    nc.sync.dma_start(out=hw1s[:, kp], in_=hw1[:, kp])
            else:
                for k in range(8):
                    nc.sync.dma_start(out=w2s[:, k, :], in_=w2[:, k, :])
                for k in range(8):
                    nc.sync.dma_start(out=hw1s[:, k, :], in_=hw1[:, k, :])
            nc.sync.dma_start(out=hw2s[:], in_=hw2[:])
            nc.sync.dma_start(out=hw3s[:], in_=hw3[:])

            # PE warm-up: dependency-free matmuls fill the initial DMA-wait
            # window and release the HAM clock throttle before real matmuls.
            if WARMUP_MMS:
                wut = wp.tile([128, ch], BF16, tag="wut")
                nc.vector.memset(wut[:], 0.0)
                wups = pp.tile([128, 2, 512], F32, tag="ps")
                for _ in range(WARMUP_MMS):
                    nc.tensor.matmul(wups[:1, 0, :ch], wut[:, 0:1],
                                     wut[:, :ch], start=True, stop=True)

            # Bias+relu epilogue, alternating ACT / DVE.
            epi_n = [0]

            def epilogue(dst, src, bias_ap, func=AF.Relu):
                if epi_n[0] % 2 == 0:
                    nc.scalar.activation(dst, src, func, bias=bias_ap)
                else:
                    nc.vector.tensor_scalar(
                        dst, src, bias_ap, 0.0, ALU.add, ALU.max)
                epi_n[0] += 1

            def layer(rhs, ktiles, wt, nm, bias, dst):
                """dst[:,m,:,:] = relu(sum_k wt[:,k,m]T @ rhs(kp,c) + b)"""
                npair = ktiles // 2
                for m in range(nm):
                    t01 = pp.tile([128, 2, 512], F32, tag="ps")
                    t23 = pp.tile([128, 2, 512], F32, tag="ps")
                    t45 = pp.tile([128, 2, 512], F32, tag="ps")
                    tt = (t01, t23, t45)
                    for kp in range(npair):
                        if USE_SWI:
                            lhs = wt[:, kp, m]
                        else:
                            lhs = wt[:, 2 * kp:2 * kp + 2,
                                     m * 128:(m + 1) * 128]
                        for c in range(n_chunks):
                            nc.tensor.matmul(
                                tt[c // 2][:, c % 2, :ch], lhs, rhs(kp, c),
                                start=(kp == 0), stop=(kp == npair - 1),
                                perf_mode=SWI if USE_SWI else DR)
                    for j in range(3):
                        epilogue(dst[:, m, 2 * j:2 * j + 2, :],
                                 tt[j][:, :, :ch], bias[:, m:m + 1])

            def hslice(t):
                return lambda kp, c: t[:, 2 * kp:2 * kp + 2, c, :]

            layer(lambda kp, c: xts[:, c, :, :], 2, w1s, 8, b1s, h1s)
            layer(hslice(h1s), 8, w2s, 8, b2s, h2s)      # L2: 1024 -> 1024
            layer(hslice(h2s), 8, hw1s, 4, hb1s, a1s)    # HL1: 1024 -> 512
            layer(hslice(a1s), 4, hw2s, 2, hb2s, a2s)    # HL2: 512 -> 256

            # HL3: 256 -> 1, sigmoid. DoubleRow with M=1; 2 chunks per psum
            # tile (partition 0 of each bank), one sigmoid per 2 chunks.
            for j in range(n_chunks // 2):
                psl = pp.tile([128, 2, 512], F32, tag="ps")
                for c2 in range(2):
                    c = 2 * j + c2
                    nc.tensor.matmul(psl[0:1, c2, :ch], hw3s[:, :, 0:1],
                                     a2s[:, :, c, :], start=True, stop=True,
                                     perf_mode=DR)
                ot = op.tile([1, 2, ch], F32, tag="ot")
                nc.scalar.activation(ot[:, :, :], psl[0:1, :, :ch],
                                     AF.Sigmoid, bias=hb3s[:1, :1])
                nc.sync.dma_start(out=out[:, 2 * j * ch:(2 * j + 2) * ch],
                                  in_=ot[:, :, :])

    removed, kept = _dedup_ldweights(nc)
    nc.compile()
    return nc


def _get_nc(cap=CAP):
    if cap not in _nc_cache:
        _nc_cache[cap] = _build(cap)
    return _nc_cache[cap]


def _q8(v):
    return np.clip(v, -240.0, 240.0).astype(NPF8)


def _tile_k(w, ktiles):
    """[K, M] -> [128, ktiles, M] fp8 with K = ktiles*128, K idx = k*128+p."""
    k, m = w.shape
    assert k == ktiles * 128
    return np.ascontiguousarray(
        _q8(w.reshape(ktiles, 128, m).transpose(1, 0, 2)))


def _swi_pack(wt):
    """[128, kt, nm*128] fp8 -> [128, kt/2, nm, 256] SwInterleave layout.

    Per (k-pair, m-tile): columns reversed, A/B planes interleaved per
    column: out[:, kp, m, 2j+i] = wt[:, 2kp+i, m*128 + (127-j)].
    """
    p, kt, M = wt.shape
    nm = M // 128
    w5 = wt.reshape(128, kt // 2, 2, nm, 128)[..., ::-1]
    return np.ascontiguousarray(
        w5.transpose(0, 1, 3, 4, 2).reshape(128, kt // 2, nm, 2, 128))


def _tile_b(b):
    """[M] -> [128, M/128] f32; column m holds bias for m-tile m."""
    m = b.shape[0]
    return np.ascontiguousarray(b.reshape(m // 128, 128).T.astype(np.float32))


def _make_in_maps(inputs):
    x = np.asarray(inputs["x"], dtype=np.float32)
    ff = np.asarray(inputs["feature_flags"]).astype(np.int64)
    idx = ff[:, 0] * 2 + ff[:, 1]

    W1 = np.asarray(inputs["W1"], np.float32)
    b1 = np.asarray(inputs["b1"], np.float32)
    W2 = np.asarray(inputs["W2"], np.float32)
    b2 = np.asarray(inputs["b2"], np.float32)
    HW1 = np.asarray(inputs["HW1"], np.float32)
    Hb1 = np.asarray(inputs["Hb1"], np.float32)
    HW2 = np.asarray(inputs["HW2"], np.float32)
    Hb2 = np.asarray(inputs["Hb2"], np.float32)
    HW3 = np.asarray(inputs["HW3"], np.float32)
    Hb3 = np.asarray(inputs["Hb3"], np.float32)

    # Row assignment: combo c -> cores 2c, 2c+1.
    row_sets = []
    for c in range(C):
        rows = np.nonzero(idx == c)[0]
        h = (len(rows) + 1) // 2
        row_sets.append(rows[:h])
        row_sets.append(rows[h:])
    max_shard = max(len(r) for r in row_sets)
    # cap = smallest multiple of 32*N_CHUNKS that fits every shard
    step = 32 * N_CHUNKS
    cap = max(CAP, -(-max_shard // step) * step)

    w1t = _tile_k(W1, 2)
    w2t = _tile_k(W2, 8)
    hw1t = [_tile_k(HW1[c], 8) for c in range(C)]
    hw2t = [_tile_k(HW2[c], 4) for c in range(C)]
    if USE_SWI:
        w1t = _swi_pack(w1t)
        w2t = _swi_pack(w2t)
        hw1t = [_swi_pack(t) for t in hw1t]
        hw2t = [_swi_pack(t) for t in hw2t]
    hw3t = []
    for c in range(C):
        t = np.zeros((128, 2, 16), NPF8)
        t[:, :, 0] = _q8(HW3[c][:, 0].reshape(2, 128).T)
        hw3t.append(t)
    cstt = []
    for c in range(C):
        cst = np.zeros((128, 23), np.float32)
        cst[:, 0:8] = _tile_b(b1)
        cst[:, 8:16] = _tile_b(b2)
        cst[:, 16:20] = _tile_b(Hb1[c])
        cst[:, 20:22] = _tile_b(Hb2[c])
        cst[:, 22] = np.float32(Hb3[c][0])
        cstt.append(cst)

    n_chunks = N_CHUNKS
    in_maps = []
    for d, rows in enumerate(row_sets):
        c = d // 2
        n = len(rows)
        ch = cap // n_chunks
        xt = np.zeros((128, 2, cap), NPF8)
        if n:
            xt[:, :, :n] = _q8(x[rows].T.reshape(2, 128, n).transpose(
                1, 0, 2))
        # [128, 2, cap] -> [128, n_chunks, 2, ch] (chunk-major)
        xt = np.ascontiguousarray(
            xt.reshape(128, 2, n_chunks, ch).transpose(0, 2, 1, 3))
        in_maps.append({
            "xT": xt,
            "w1": w1t, "w2": w2t,
            "hw1": hw1t[c], "hw2": hw2t[c], "hw3": hw3t[c],
            "consts": cstt[c],
        })

    return in_maps, row_sets, cap


def kernel(**inputs):
    global _last_results
    in_maps, row_sets, cap = _make_in_maps(inputs)
    nc = _get_nc(cap)
    res = run_bass_kernel_spmd(nc, in_maps, core_ids=list(range(NCORES)))
    _last_results = res

    out = np.empty(B, np.float32)
    for d, rows in enumerate(row_sets):
        if len(rows):
            out[rows] = res.results[d]["out"][0, :len(rows)]
    return out
